# revision 32
# baseline (speedup 1.0000x reference)
"""Sliding-window decoder layer on 8 trn2 NeuronCores.

Sharding: sequence-parallel. T=4096 is split into 8 blocks of 512 (= WINDOW),
one per core. The sliding-window mask (each query attends the previous 512
keys, inclusive) means core i only needs k/v for rows [i*512-512, (i+1)*512):
its own block plus the previous one. Each core recomputes that k/v halo
locally from x, so there is no cross-core communication at all; weights are
replicated. This beats the Megatron-TP hint here because two 16MB all-reduces
(~380us) would dominate the ~300us of per-core compute.

Math identities used (exact up to the tiny fp32 eps):
- The x-rmsnorm is a positive row scale and rmsnorm is scale-invariant, so it
  cancels inside the per-head q/k rmsnorm; q/k are projected straight from x
  and only the v path applies the x-rmsnorm row scale.
- The mlp rmsnorm row scale commutes through relu^2 (relu(a*z)^2 = a^2
  relu(z)^2, a>0), so it is applied once at the end:
  out = x2 + rstd2^2 * (relu(x2 @ w1T)^2 @ w2sT).

All matmuls run as float32r (fp32 bytes, ~1e-4 matmul relative error, 4x the
plain-fp32 tensor-engine throughput). The sliding window mask is applied with
one gpsimd affine_select per 128-row kv tile (the band condition degenerates
to a single inequality per tile). Core 0 has no left halo: its halo x is
zero and a per-core "vones" input zeroes the softmax-denominator column for
the invalid kv tiles.
"""

import threading

import numpy as np

DIM = 1024
MLP_DIM = 4096
T = 4096
W = 512
HD = 64
NH = 16
NCORES = 8
EPS = 1.1920929e-07
ROPE_BASE = 10000.0

_lock = threading.Lock()
_program = None


def _emit(nc, tc, d, mybir, phases=99):
    """Emit the whole per-core program. d: dict of dram handles."""
    from contextlib import ExitStack

    F32 = mybir.dt.float32
    F32R = mybir.dt.float32r
    AF = mybir.ActivationFunctionType
    OP = mybir.AluOpType

    top = ExitStack()
    persist = top.enter_context(tc.tile_pool(name="persist", bufs=1))
    psA = top.enter_context(tc.tile_pool(name="psA", bufs=3, space="PSUM"))

    # small constants (host-provided; partial-partition memsets fail ISA checks)
    E2 = persist.tile([128, 2], F32R)
    nc.gpsimd.dma_start(E2[:], d["E2"].ap()[:])
    E2b = persist.tile([2, 128], F32R)
    nc.gpsimd.dma_start(E2b[:], d["E2b"].ap()[:])
    Id128r = persist.tile([128, 128], F32R)
    nc.gpsimd.dma_start(Id128r[:], d["Id128r"].ap()[:])
    Pswap = persist.tile([128, 128], F32R)
    nc.gpsimd.dma_start(Pswap[:], d["Pswap"].ap()[:])
    ones64 = persist.tile([128, 64], F32R)
    nc.gpsimd.dma_start(ones64[:], d["ones64"].ap()[:])
    BF16 = mybir.dt.bfloat16
    Id128 = persist.tile([128, 128], BF16)
    nc.gpsimd.dma_start(Id128[:], d["Id128b"].ap()[:])
    epsc = persist.tile([128, 1], F32)
    nc.gpsimd.memset(epsc[:], EPS)
    rstd2sq = persist.tile([128, 4], F32)



    # scope A: attention + o-projection
    sA = ExitStack()
    apool = sA.enter_context(tc.tile_pool(name="attnA", bufs=1))
    xTh = apool.tile([128, 8, 1024], F32R)
    for kk in range(8):
        nc.sync.dma_start(xTh[:, kk, :], d["xTh"].ap()[:, kk, :])
    attnT = apool.tile([128, 8, W], F32R)

    # ---------------- attention inner scope ----------------
    sI = ExitStack()
    ipool = sI.enter_context(tc.tile_pool(name="attnI", bufs=1))
    w_qk = sI.enter_context(tc.tile_pool(name="wqk", bufs=2))
    w_v = sI.enter_context(tc.tile_pool(name="wv", bufs=1))
    t1 = sI.enter_context(tc.tile_pool(name="t1", bufs=3))
    tsw = sI.enter_context(tc.tile_pool(name="tsw", bufs=2))
    t2 = sI.enter_context(tc.tile_pool(name="t2", bufs=1))
    ptp = sI.enter_context(tc.tile_pool(name="ptp", bufs=3))
    psPV = sI.enter_context(tc.tile_pool(name="psPV", bufs=2, space="PSUM"))
    psS = sI.enter_context(tc.tile_pool(name="psS", bufs=3, space="PSUM"))

    cosr = ipool.tile([128, 1024], F32)
    nc.gpsimd.dma_start(cosr[:], d["cosr"].ap()[:])
    sinr = ipool.tile([128, 1024], F32)
    nc.gpsimd.dma_start(sinr[:], d["sinr"].ap()[:])
    vpad = ipool.tile([128, 8, 2], F32R)
    nc.gpsimd.dma_start(vpad[:], d["vpad"].ap()[:])
    maskb = ipool.tile([128, 8, W], BF16)
    nc.gpsimd.dma_start(maskb[:], d["maskb"].ap()[:])

    qT = ipool.tile([128, 8, W], F32R)
    kT = ipool.tile([128, 8, 1024], F32R)
    vv = ipool.tile([128, 8, 16, 66], F32R)
    nc.vector.tensor_copy(vv[:, :, :, 64:66],
                          vpad[:, :, None, :].to_broadcast((128, 8, 16, 2)))

    # ---- rstd_x (v path) ----
    ssx = ipool.tile([128, 8], F32)
    for rt in range(8):
        xrow = t2.tile([128, 1024], F32, tag="xrow")
        nc.scalar.dma_start(xrow[:], d["xh"].ap()[128 * rt:128 * (rt + 1), :])
        nc.scalar.activation(xrow[:], xrow[:], AF.Square,
                             accum_out=ssx[:, rt:rt + 1])
    rstdx = ipool.tile([128, 8], F32)
    nc.vector.tensor_scalar(rstdx[:], ssx[:], 1.0 / DIM, EPS, OP.mult, OP.add)
    nc.scalar.activation(rstdx[:], rstdx[:], AF.Sqrt)
    nc.vector.reciprocal(rstdx[:], rstdx[:])

    if phases < 1:
        sI.close(); sA.close(); top.close(); return
    # ---- qkv projection ----
    for t in range(8):  # q tiles (own 512 rows)
        wq = w_qk.tile([128, 8, 128], F32R, tag="wqk")
        nc.sync.dma_start(wq[:], d["qkv_wtl"].ap()[t])
        ps = psA.tile([128, W], F32, tag="mm")
        for kk in range(8):
            nc.tensor.matmul(ps[:], wq[:, kk, :], xTh[:, kk, 512:1024],
                             start=(kk == 0), stop=(kk == 7))
        nc.vector.tensor_copy(qT[:, t, :], ps[:])
    for t in range(8):  # k tiles (halo 1024 rows)
        wk = w_qk.tile([128, 8, 128], F32R, tag="wqk")
        nc.sync.dma_start(wk[:], d["qkv_wtl"].ap()[8 + t])
        for ch in range(2):
            ps = psA.tile([128, W], F32, tag="mm")
            for kk in range(8):
                nc.tensor.matmul(ps[:], wk[:, kk, :],
                                 xTh[:, kk, 512 * ch:512 * (ch + 1)],
                                 start=(kk == 0), stop=(kk == 7))
            nc.vector.tensor_copy(kT[:, t, 512 * ch:512 * (ch + 1)], ps[:])
    for ch in range(2):  # v, natural layout, x-rmsnorm row scale applied
        wv = w_v.tile([128, 8, 512], F32R, tag="wv")
        nc.sync.dma_start(wv[:], d["wv_tl"].ap()[ch])
        for rt in range(8):
            ps = psA.tile([128, W], F32, tag="mm")
            for kk in range(8):
                nc.tensor.matmul(ps[:], xTh[:, kk, 128 * rt:128 * (rt + 1)],
                                 wv[:, kk, :], start=(kk == 0), stop=(kk == 7))
            nc.vector.tensor_scalar_mul(vv[:, rt, 8 * ch:8 * (ch + 1), 0:64],
                                        ps[:], rstdx[:, rt:rt + 1])

    # ---- per-head rmsnorm + rope ----
    def headnorm_rope(buf, t, width, cs_off):
        for ch in range(width // 512):
            sl = slice(512 * ch, 512 * (ch + 1))
            sq = t1.tile([128, 512], F32R, tag="sq")
            nc.vector.tensor_tensor(sq[:], buf[:, t, sl], buf[:, t, sl], OP.mult)
            ssp = psS.tile([128, 512], F32, tag="small")
            nc.tensor.matmul(ssp[0:2, :], E2[:], sq[:], start=True, stop=True)
            rstd = t1.tile([2, 512], F32R, tag="rstd")
            nc.scalar.activation(rstd[:], ssp[0:2, :], AF.Sqrt,
                                 scale=1.0 / HD, bias=epsc[0:2, :])
            with nc.allow_low_precision(reason="fp32r is fp32 in memory"):
                nc.vector.reciprocal(rstd[:], rstd[:])
            bcp = psS.tile([128, 512], F32, tag="small")
            nc.tensor.matmul(bcp[:], E2b[:], rstd[:], start=True, stop=True)
            nc.vector.tensor_tensor(buf[:, t, sl], buf[:, t, sl], bcp[:], OP.mult)
        # rope: y = qn*cos + swap32(qn)*sin_signed; swap via PE permutation
        sw = tsw.tile([128, 1024], F32R, tag="sw")
        for ch in range(width // 512):
            sl = slice(512 * ch, 512 * (ch + 1))
            swp = psS.tile([128, 512], F32, tag="small")
            nc.tensor.matmul(swp[:], Pswap[:], buf[:, t, sl], start=True, stop=True)
            nc.vector.tensor_tensor(sw[:, sl], swp[:],
                                    sinr[:, cs_off + 512 * ch:cs_off + 512 * (ch + 1)],
                                    OP.mult)
        nc.vector.tensor_tensor(buf[:, t, 0:width], buf[:, t, 0:width],
                                cosr[:, cs_off:cs_off + width], OP.mult)
        nc.vector.tensor_tensor(buf[:, t, 0:width], buf[:, t, 0:width],
                                sw[:, 0:width], OP.add)

    if phases >= 2:
        for t in range(8):
            headnorm_rope(qT, t, W, 512)
            headnorm_rope(kT, t, 1024, 0)
    if phases < 3:
        sI.close(); sA.close(); top.close(); return

    # ---- sliding-window attention ----
    for t in range(8):
        for h in range(2):
            b = 64 * h
            hg = 2 * t + h
            pvp = psPV.tile([66, W], F32, tag="pv")
            for j in range(8):
                scp = psA.tile([128, W], F32, tag="mm")
                nc.tensor.matmul(scp[:], Id128[:], maskb[:, j, :],
                                 start=True, stop=False)
                nc.tensor.matmul(scp[:], kT[b:b + 64, t, 128 * j:128 * (j + 1)],
                                 qT[b:b + 64, t, :], start=False, stop=True)
                pt = ptp.tile([128, W], F32R, tag="pt")
                nc.scalar.activation(pt[:], scp[:], AF.Exp, scale=0.125)
                nc.tensor.matmul(pvp[:], vv[:, j, hg, :], pt[:],
                                 start=(j == 0), stop=(j == 7))
            rs = t1.tile([66, W], F32R, tag="rs")
            with nc.allow_low_precision(reason="fp32r is fp32 in memory"):
                nc.vector.reciprocal(rs[64:65, :], pvp[64:65, :])
            bc2 = psS.tile([128, W], F32, tag="small")
            nc.tensor.matmul(bc2[0:64, :], ones64[64:65, 0:64], rs[64:65, :],
                             start=True, stop=True)
            nc.vector.tensor_copy(attnT[b:b + 64, t, :], pvp[0:64, :])
            nc.vector.tensor_tensor(attnT[b:b + 64, t, :], attnT[b:b + 64, t, :],
                                    bc2[0:64, :], OP.mult)

    sI.close()  # free attention inner tiles

    if phases < 4:
        sA.close(); top.close(); return
    # ---- MLP scope opens first so x2T can live in SBUF across phases ----
    sM = ExitStack()
    mpool = sM.enter_context(tc.tile_pool(name="mlp", bufs=1))
    mw = sM.enter_context(tc.tile_pool(name="mw", bufs=4))
    psD = sM.enter_context(tc.tile_pool(name="psD", bufs=4, space="PSUM"))
    x2T = mpool.tile([128, 8, W], F32R)

    # -------- o-projection (oT orientation only) -> x2T ------
    sO = ExitStack()
    wo_p = sO.enter_context(tc.tile_pool(name="wo", bufs=4))
    for jt in range(8):
        wo = wo_p.tile([128, 8, 128], F32R, tag="wo")
        nc.sync.dma_start(wo[:], d["wo_tl"].ap()[jt])
        ps = psA.tile([128, W], F32, tag="mm")
        for kk in range(8):
            nc.tensor.matmul(ps[:], wo[:, kk, :], attnT[:, kk, :],
                             start=(kk == 0), stop=(kk == 7))
        nc.vector.tensor_tensor(x2T[:, jt, :], ps[:], xTh[:, jt, 512:1024], OP.add)
    sO.close()

    if phases < 5:
        sM.close(); sA.close(); top.close(); return
    # ---------------- MLP ----------------
    # x2 natural layout via PE block transposes of x2T (exact up to fp32r)
    x2n = mpool.tile([128, 4, 1024], F32)
    ss2 = mpool.tile([128, 4], F32)
    scr2 = mpool.tile([128, 1024], F32)
    for qt in range(4):
        for jt in range(8):
            tp = psA.tile([128, W], F32R, tag="mm")
            nc.tensor.transpose(tp[:, 0:128], x2T[:, jt, 128 * qt:128 * (qt + 1)],
                                Id128r[:])
            nc.vector.tensor_copy(x2n[:, qt, 128 * jt:128 * (jt + 1)], tp[:, 0:128])
        nc.scalar.activation(scr2[:], x2n[:, qt, :], AF.Square,
                             accum_out=ss2[:, qt:qt + 1])
    nc.vector.tensor_scalar(rstd2sq[:], ss2[:], 1.0 / DIM, EPS, OP.mult, OP.add)
    nc.vector.reciprocal(rstd2sq[:], rstd2sq[:])

    outsb = mpool.tile([128, 4, 1024], F32)
    nc.vector.memset(outsb[:], 0.0)
    hp = sM.enter_context(tc.tile_pool(name="hp", bufs=10))
    w2p = sM.enter_context(tc.tile_pool(name="w2p", bufs=5))

    for g in range(4):
        hts = []
        for ml in range(8):
            mt = 8 * g + ml
            w1t = mw.tile([128, 8, 128], F32R, tag="w1t")
            nc.sync.dma_start(w1t[:], d["w1_tl"].ap()[mt])
            ps = psA.tile([128, W], F32, tag="mm")
            for kk in range(8):
                nc.tensor.matmul(ps[:], w1t[:, kk, :], x2T[:, kk, :],
                                 start=(kk == 0), stop=(kk == 7))
            ht = hp.tile([128, W], F32R, tag="ht", name=f"ht{g}_{ml}")
            nc.scalar.activation(ht[:], ps[:], AF.Relu)
            nc.vector.tensor_tensor(ht[:], ht[:], ht[:], OP.mult)
            hts.append(ht)
        w2ts = []
        for half in range(2):
            pss = [psD.tile([128, W], F32, tag="down", name=f"dn{g}_{half}_{i}")
                   for i in range(4)]
            for kl in range(8):
                if half == 0 and kl % 2 == 0:
                    w2t2 = w2p.tile([128, 2, 1024], F32R, tag="w2t", name=f"w2t{g}_{kl}")
                    nc.sync.dma_start(w2t2[:], d["w2_tl"].ap()[(8 * g + kl) // 2])
                    w2ts.append(w2t2)
                w2t = w2ts[kl // 2][:, kl % 2, :]
                for qt in range(4):
                    nc.tensor.matmul(pss[qt][:], hts[kl][:, 128 * qt:128 * (qt + 1)],
                                     w2t[:, 512 * half:512 * (half + 1)],
                                     start=(kl == 0), stop=(kl == 7))
            for qt in range(4):
                nc.vector.tensor_tensor(
                    outsb[:, qt, 512 * half:512 * (half + 1)],
                    outsb[:, qt, 512 * half:512 * (half + 1)], pss[qt][:], OP.add)

    for qt in range(4):
        for half in range(2):
            sl = slice(512 * half, 512 * (half + 1))
            nc.vector.scalar_tensor_tensor(
                outsb[:, qt, sl], outsb[:, qt, sl], rstd2sq[:, qt:qt + 1],
                x2n[:, qt, sl], OP.mult, OP.add)
    for qt in range(4):
        nc.scalar.dma_start(d["out"].ap()[128 * qt:128 * (qt + 1), :], outsb[:, qt, :])
    sM.close()
    sA.close()
    top.close()


def _build_program(phases=99, reps=1):
    import concourse.bacc as bacc
    import concourse.tile as tile
    import concourse.mybir as mybir

    F32 = mybir.dt.float32
    F32R = mybir.dt.float32r

    nc = bacc.Bacc("TRN2")
    d = {
        "xh": nc.declare_dram_parameter("xh", [1024, DIM], F32, isOutput=False),
        "xTh": nc.declare_dram_parameter("xTh", [128, 8, 1024], F32R, isOutput=False),
        "cosr": nc.declare_dram_parameter("cosr", [128, 1024], F32, isOutput=False),
        "sinr": nc.declare_dram_parameter("sinr", [128, 1024], F32, isOutput=False),
        "vpad": nc.declare_dram_parameter("vpad", [128, 8, 2], F32R, isOutput=False),
        "maskb": nc.declare_dram_parameter("maskb", [128, 8, W], mybir.dt.bfloat16, isOutput=False),
        "ones64": nc.declare_dram_parameter("ones64", [128, 64], F32R, isOutput=False),
        "Id128b": nc.declare_dram_parameter("Id128b", [128, 128], mybir.dt.bfloat16, isOutput=False),
        "E2": nc.declare_dram_parameter("E2", [128, 2], F32R, isOutput=False),
        "E2b": nc.declare_dram_parameter("E2b", [2, 128], F32R, isOutput=False),
        "Id128r": nc.declare_dram_parameter("Id128r", [128, 128], F32R, isOutput=False),
        "Pswap": nc.declare_dram_parameter("Pswap", [128, 128], F32R, isOutput=False),
        "qkv_wtl": nc.declare_dram_parameter("qkv_wtl", [16, 128, 8, 128], F32R, isOutput=False),
        "wv_tl": nc.declare_dram_parameter("wv_tl", [2, 128, 8, 512], F32R, isOutput=False),
        "wo_tl": nc.declare_dram_parameter("wo_tl", [8, 128, 8, 128], F32R, isOutput=False),
        "w1_tl": nc.declare_dram_parameter("w1_tl", [32, 128, 8, 128], F32R, isOutput=False),
        "w2_tl": nc.declare_dram_parameter("w2_tl", [16, 128, 2, 1024], F32R, isOutput=False),
        "out": nc.declare_dram_parameter("out", [W, DIM], F32, isOutput=True),
    }
    with tile.TileContext(nc) as tc:
        for _ in range(reps):
            _emit(nc, tc, d, mybir, phases=phases)
    nc.compile()
    return nc


def _host_prep(x, qkv_w, o_w, o_scale, w1, w2, w2_scale):
    x2d = np.ascontiguousarray(x.reshape(T, DIM).astype(np.float32))
    xT = np.ascontiguousarray(x2d.T)

    qkv_wT = qkv_w.astype(np.float32).T                                  # [1024, 3072]
    o_wsT = (o_w * o_scale[:, None]).astype(np.float32).T                # [1024, 1024]
    w1T = w1.astype(np.float32).T                                        # [1024, 4096]
    w2sT = (w2 * w2_scale[:, None]).astype(np.float32).T
    w2_tl = np.ascontiguousarray(
        w2sT.reshape(16, 2, 128, 1024).transpose(0, 2, 1, 3))

    def tile_cols(wT, ncols):
        # [1024, C] -> [C//ncols, 128, 8, ncols] with [ct, p, kk, m] layout
        kk8, p, ct = 8, 128, wT.shape[1] // ncols
        return np.ascontiguousarray(
            wT.reshape(kk8, p, ct, ncols).transpose(2, 1, 0, 3))

    qkv_wtl = tile_cols(qkv_wT[:, :2048], 128)        # [16, 128, 8, 128]
    wv_tl = tile_cols(qkv_wT[:, 2048:], 512)          # [2, 128, 8, 512]
    wo_tl = tile_cols(o_wsT, 128)                     # [8, 128, 8, 128]
    w1_tl = tile_cols(w1T, 128)                       # [32, 128, 8, 128]

    freqs = (1.0 / ROPE_BASE) ** np.linspace(0.0, 1.0, HD // 4, dtype=np.float32)
    freqs_d = np.concatenate([freqs, np.zeros(HD // 4, dtype=np.float32)])
    theta = np.arange(T, dtype=np.float32)[:, None] * freqs_d[None, :]   # [T, 32]
    cosT = np.cos(theta).T.astype(np.float32)                            # [32, T]
    sinT = np.sin(theta).T.astype(np.float32)
    cos_rep = np.concatenate([cosT, cosT, cosT, cosT], axis=0)           # [128, T]
    sin_sgn = np.concatenate([sinT, -sinT, sinT, -sinT], axis=0)

    in_maps = []
    for c in range(NCORES):
        r0 = c * W - W
        xh = np.zeros((1024, DIM), dtype=np.float32)
        xTh = np.zeros((DIM, 1024), dtype=np.float32)
        cosr = np.zeros((128, 1024), dtype=np.float32)
        sinr = np.zeros((128, 1024), dtype=np.float32)
        lo = max(r0, 0)
        off = lo - r0
        xh[off:1024] = x2d[lo:r0 + 1024]
        xTh[:, off:1024] = xT[:, lo:r0 + 1024]
        xTh_t = np.ascontiguousarray(xTh.reshape(8, 128, 1024).transpose(1, 0, 2))
        cosr[:, off:1024] = cos_rep[:, lo:r0 + 1024]
        sinr[:, off:1024] = sin_sgn[:, lo:r0 + 1024]
        import ml_dtypes
        vpadc = np.zeros((128, 8, 2), dtype=np.float32)
        vpadc[:, :, 0] = 1.0
        if c == 0:
            vpadc[:, 0:4, 0] = 0.0
        kvl = np.arange(1024)[:, None]
        ql = np.arange(W)[None, :]
        valid = (kvl > ql) & (kvl <= ql + W)
        mb = np.where(valid, 0.0, -240.0).astype(np.float32)
        maskbc = np.ascontiguousarray(
            mb.reshape(8, 128, W).transpose(1, 0, 2)).astype(ml_dtypes.bfloat16)
        E2c = (np.arange(128)[:, None] // 64 == np.arange(2)[None, :]).astype(np.float32)
        E2bc = np.ascontiguousarray(E2c.T)
        idc = np.eye(128, dtype=np.float32)
        psw = np.zeros((128, 128), dtype=np.float32)
        psw[np.arange(128), np.arange(128) ^ 32] = 1.0
        psw = np.ascontiguousarray(psw.T)  # lhsT layout: out[m] = in[m ^ 32]
        in_maps.append({
            "xh": xh,
            "xTh": xTh_t,
            "cosr": np.ascontiguousarray(cosr),
            "sinr": np.ascontiguousarray(sinr),
            "vpad": vpadc,
            "maskb": maskbc,
            "ones64": np.ones((128, 64), dtype=np.float32),
            "Id128b": np.eye(128, dtype=np.float32).astype(ml_dtypes.bfloat16),
            "E2": E2c,
            "E2b": E2bc,
            "Id128r": idc,
            "Pswap": psw,
            "qkv_wtl": qkv_wtl,
            "wv_tl": wv_tl,
            "wo_tl": wo_tl,
            "w1_tl": w1_tl,
            "w2_tl": w2_tl,
        })
    return in_maps


def kernel(x, qkv_w, o_w, o_scale, w1, w2, w2_scale):
    from concourse.bass_utils import run_bass_kernel_spmd

    global _program
    with _lock:
        if _program is None:
            _program = _build_program()
    in_maps = _host_prep(x, qkv_w, o_w, o_scale, w1, w2, w2_scale)
    res = run_bass_kernel_spmd(_program, in_maps, core_ids=list(range(NCORES)))
    out = np.concatenate([res.results[c]["out"] for c in range(NCORES)], axis=0)
    return out.reshape(1, T, DIM).astype(np.float32)


# revision 39
# speedup vs baseline: 23821.4803x; 23821.4803x over previous
"""Sliding-window decoder layer on 8 trn2 NeuronCores.

Sharding: sequence-parallel. T=4096 is split into 8 blocks of 512 (= WINDOW),
one per core. The sliding-window mask (each query attends the previous 512
keys, inclusive) means core i only needs k/v for rows [i*512-512, (i+1)*512):
its own block plus the previous one. Each core recomputes that k/v halo
locally from x, so there is no cross-core communication at all; weights are
replicated. This beats the Megatron-TP hint here because two 16MB all-reduces
(~380us) would dominate the ~300us of per-core compute.

Math identities used (exact up to the tiny fp32 eps):
- The x-rmsnorm is a positive row scale and rmsnorm is scale-invariant, so it
  cancels inside the per-head q/k rmsnorm; q/k are projected straight from x
  and only the v path applies the x-rmsnorm row scale.
- The mlp rmsnorm row scale commutes through relu^2 (relu(a*z)^2 = a^2
  relu(z)^2, a>0), so it is applied once at the end:
  out = x2 + rstd2^2 * (relu(x2 @ w1T)^2 @ w2sT).

All matmuls run as float32r (fp32 bytes, ~1e-4 matmul relative error, 4x the
plain-fp32 tensor-engine throughput). The sliding window mask is applied with
one gpsimd affine_select per 128-row kv tile (the band condition degenerates
to a single inequality per tile). Core 0 has no left halo: its halo x is
zero and a per-core "vones" input zeroes the softmax-denominator column for
the invalid kv tiles.
"""

import threading

import numpy as np

DIM = 1024
MLP_DIM = 4096
T = 4096
W = 512
HD = 64
NH = 16
NCORES = 8
EPS = 1.1920929e-07
ROPE_BASE = 10000.0

_lock = threading.Lock()
_program = None


def _emit(nc, tc, d, mybir, phases=99):
    """Emit the whole per-core program. d: dict of dram handles."""
    from contextlib import ExitStack

    F32 = mybir.dt.float32
    F32R = mybir.dt.float32r
    AF = mybir.ActivationFunctionType
    OP = mybir.AluOpType

    top = ExitStack()
    persist = top.enter_context(tc.tile_pool(name="persist", bufs=1))
    psA = top.enter_context(tc.tile_pool(name="psA", bufs=3, space="PSUM"))

    # small constants (host-provided); their DMAs are issued after the first
    # weight DMAs so the first matmul's inputs hit the queues first
    BF16 = mybir.dt.bfloat16
    E2 = persist.tile([128, 2], F32R)
    E2b = persist.tile([2, 128], F32R)
    Id128r = persist.tile([128, 128], F32R)
    Pswap = persist.tile([128, 128], F32R)
    ones64 = persist.tile([128, 64], F32R)
    Id128 = persist.tile([128, 128], BF16)
    epsc = persist.tile([128, 1], F32)
    nc.gpsimd.memset(epsc[:], EPS)
    rstd2sq = persist.tile([128, 4], F32)



    # scope A: attention + o-projection
    sA = ExitStack()
    apool = sA.enter_context(tc.tile_pool(name="attnA", bufs=1))
    xTh = apool.tile([128, 8, 1024], F32R)
    for kk in range(8):
        eng = (nc.scalar, nc.gpsimd)[kk % 2]
        eng.dma_start(xTh[:, kk, :], d["xTh"].ap()[:, kk, :])
    attnT = apool.tile([128, 8, W], F32R)

    # ---------------- attention inner scope ----------------
    sI = ExitStack()
    ipool = sI.enter_context(tc.tile_pool(name="attnI", bufs=1))
    w_qk = sI.enter_context(tc.tile_pool(name="wqk", bufs=2))
    w_v = sI.enter_context(tc.tile_pool(name="wv", bufs=1))
    t1 = sI.enter_context(tc.tile_pool(name="t1", bufs=3))
    tsw = sI.enter_context(tc.tile_pool(name="tsw", bufs=2))
    t2 = sI.enter_context(tc.tile_pool(name="t2", bufs=1))
    ptp = sI.enter_context(tc.tile_pool(name="ptp", bufs=3))
    psPV = sI.enter_context(tc.tile_pool(name="psPV", bufs=2, space="PSUM"))
    psS = sI.enter_context(tc.tile_pool(name="psS", bufs=3, space="PSUM"))

    cosr = ipool.tile([128, 1024], F32)
    sinr = ipool.tile([128, 1024], F32)
    vpad = ipool.tile([128, 8, 2], F32R)
    maskb = ipool.tile([128, 8, W], BF16)
    qT = ipool.tile([128, 8, W], F32R)
    kT = ipool.tile([128, 8, 1024], F32R)
    vv = ipool.tile([128, 8, 16, 66], F32R)
    ssx = ipool.tile([128, 8], F32)
    rstdx = ipool.tile([128, 8], F32)

    if phases < 1:
        sI.close(); sA.close(); top.close(); return
    # ---- qkv projection ----
    for t in range(8):  # q tiles (own 512 rows)
        wq = w_qk.tile([128, 8, 128], F32R, tag="wqk")
        nc.sync.dma_start(wq[:], d["qkv_wtl"].ap()[t])
        ps = psA.tile([128, W], F32, tag="mm")
        for kk in range(8):
            nc.tensor.matmul(ps[:], wq[:, kk, :], xTh[:, kk, 512:1024],
                             start=(kk == 0), stop=(kk == 7))
        nc.vector.tensor_copy(qT[:, t, :], ps[:])
    # deferred constant/table loads + rstd_x (first needed at the v-scale)
    nc.gpsimd.dma_start(E2[:], d["E2"].ap()[:])
    nc.gpsimd.dma_start(E2b[:], d["E2b"].ap()[:])
    nc.gpsimd.dma_start(Pswap[:], d["Pswap"].ap()[:])
    nc.gpsimd.dma_start(cosr[:], d["cosr"].ap()[:])
    nc.gpsimd.dma_start(sinr[:], d["sinr"].ap()[:])
    nc.gpsimd.dma_start(vpad[:], d["vpad"].ap()[:])
    nc.gpsimd.dma_start(maskb[:], d["maskb"].ap()[:])
    nc.gpsimd.dma_start(ones64[:], d["ones64"].ap()[:])
    nc.gpsimd.dma_start(Id128[:], d["Id128b"].ap()[:])
    nc.gpsimd.dma_start(Id128r[:], d["Id128r"].ap()[:])
    nc.vector.tensor_copy(vv[:, :, :, 64:66],
                          vpad[:, :, None, :].to_broadcast((128, 8, 16, 2)))
    for rt in range(8):
        xrow = t2.tile([128, 1024], F32, tag="xrow")
        nc.scalar.dma_start(xrow[:], d["xh"].ap()[128 * rt:128 * (rt + 1), :])
        nc.scalar.activation(xrow[:], xrow[:], AF.Square,
                             accum_out=ssx[:, rt:rt + 1])
    nc.vector.tensor_scalar(rstdx[:], ssx[:], 1.0 / DIM, EPS, OP.mult, OP.add)
    nc.scalar.activation(rstdx[:], rstdx[:], AF.Sqrt)
    nc.vector.reciprocal(rstdx[:], rstdx[:])

    for t in range(8):  # k tiles (halo 1024 rows)
        wk = w_qk.tile([128, 8, 128], F32R, tag="wqk")
        nc.sync.dma_start(wk[:], d["qkv_wtl"].ap()[8 + t])
        for ch in range(2):
            ps = psA.tile([128, W], F32, tag="mm")
            for kk in range(8):
                nc.tensor.matmul(ps[:], wk[:, kk, :],
                                 xTh[:, kk, 512 * ch:512 * (ch + 1)],
                                 start=(kk == 0), stop=(kk == 7))
            nc.vector.tensor_copy(kT[:, t, 512 * ch:512 * (ch + 1)], ps[:])
    for ch in range(2):  # v, natural layout, x-rmsnorm row scale applied
        wv = w_v.tile([128, 8, 512], F32R, tag="wv")
        nc.sync.dma_start(wv[:], d["wv_tl"].ap()[ch])
        for rt in range(8):
            ps = psA.tile([128, W], F32, tag="mm")
            for kk in range(8):
                nc.tensor.matmul(ps[:], xTh[:, kk, 128 * rt:128 * (rt + 1)],
                                 wv[:, kk, :], start=(kk == 0), stop=(kk == 7))
            nc.vector.tensor_scalar_mul(vv[:, rt, 8 * ch:8 * (ch + 1), 0:64],
                                        ps[:], rstdx[:, rt:rt + 1])

    # ---- per-head rmsnorm + rope ----
    def headnorm_rope(buf, t, width, cs_off):
        for ch in range(width // 512):
            sl = slice(512 * ch, 512 * (ch + 1))
            sq = t1.tile([128, 512], F32R, tag="sq")
            nc.vector.tensor_tensor(sq[:], buf[:, t, sl], buf[:, t, sl], OP.mult)
            ssp = psS.tile([128, 512], F32, tag="small")
            nc.tensor.matmul(ssp[0:2, :], E2[:], sq[:], start=True, stop=True)
            rstd = t1.tile([2, 512], F32R, tag="rstd")
            nc.scalar.activation(rstd[:], ssp[0:2, :], AF.Sqrt,
                                 scale=1.0 / HD, bias=epsc[0:2, :])
            with nc.allow_low_precision(reason="fp32r is fp32 in memory"):
                nc.vector.reciprocal(rstd[:], rstd[:])
            bcp = psS.tile([128, 512], F32, tag="small")
            nc.tensor.matmul(bcp[:], E2b[:], rstd[:], start=True, stop=True)
            nc.vector.tensor_tensor(buf[:, t, sl], buf[:, t, sl], bcp[:], OP.mult)
        # rope: y = qn*cos + swap32(qn)*sin_signed; swap via PE permutation
        sw = tsw.tile([128, 1024], F32R, tag="sw")
        for ch in range(width // 512):
            sl = slice(512 * ch, 512 * (ch + 1))
            swp = psS.tile([128, 512], F32, tag="small")
            nc.tensor.matmul(swp[:], Pswap[:], buf[:, t, sl], start=True, stop=True)
            nc.vector.tensor_tensor(sw[:, sl], swp[:],
                                    sinr[:, cs_off + 512 * ch:cs_off + 512 * (ch + 1)],
                                    OP.mult)
        nc.vector.tensor_tensor(buf[:, t, 0:width], buf[:, t, 0:width],
                                cosr[:, cs_off:cs_off + width], OP.mult)
        nc.vector.tensor_tensor(buf[:, t, 0:width], buf[:, t, 0:width],
                                sw[:, 0:width], OP.add)

    if phases >= 2:
        for t in range(8):
            headnorm_rope(qT, t, W, 512)
            headnorm_rope(kT, t, 1024, 0)
    if phases < 3:
        sI.close(); sA.close(); top.close(); return

    # ---- sliding-window attention ----
    for t in range(8):
        for h in range(2):
            b = 64 * h
            hg = 2 * t + h
            pvp = psPV.tile([66, W], F32, tag="pv")
            for j in range(8):
                scp = psA.tile([128, W], F32, tag="mm")
                nc.tensor.matmul(scp[:], Id128[:], maskb[:, j, :],
                                 start=True, stop=False)
                nc.tensor.matmul(scp[:], kT[b:b + 64, t, 128 * j:128 * (j + 1)],
                                 qT[b:b + 64, t, :], start=False, stop=True)
                pt = ptp.tile([128, W], F32R, tag="pt")
                nc.scalar.activation(pt[:], scp[:], AF.Exp, scale=0.125)
                nc.tensor.matmul(pvp[:], vv[:, j, hg, :], pt[:],
                                 start=(j == 0), stop=(j == 7))
            rs = t1.tile([66, W], F32R, tag="rs")
            with nc.allow_low_precision(reason="fp32r is fp32 in memory"):
                nc.vector.reciprocal(rs[64:65, :], pvp[64:65, :])
            bc2 = psS.tile([128, W], F32, tag="small")
            nc.tensor.matmul(bc2[0:64, :], ones64[64:65, 0:64], rs[64:65, :],
                             start=True, stop=True)
            nc.vector.tensor_copy(attnT[b:b + 64, t, :], pvp[0:64, :])
            nc.vector.tensor_tensor(attnT[b:b + 64, t, :], attnT[b:b + 64, t, :],
                                    bc2[0:64, :], OP.mult)

    sI.close()  # free attention inner tiles

    if phases < 4:
        sA.close(); top.close(); return
    # ---- MLP scope opens first so x2T can live in SBUF across phases ----
    sM = ExitStack()
    mpool = sM.enter_context(tc.tile_pool(name="mlp", bufs=1))
    mw = sM.enter_context(tc.tile_pool(name="mw", bufs=4))
    psD = sM.enter_context(tc.tile_pool(name="psD", bufs=4, space="PSUM"))
    x2T = mpool.tile([128, 8, W], F32R)

    # -------- o-projection (oT orientation only) -> x2T ------
    sO = ExitStack()
    wo_p = sO.enter_context(tc.tile_pool(name="wo", bufs=4))
    for jt in range(8):
        wo = wo_p.tile([128, 8, 128], F32R, tag="wo")
        nc.sync.dma_start(wo[:], d["wo_tl"].ap()[jt])
        ps = psA.tile([128, W], F32, tag="mm")
        for kk in range(8):
            nc.tensor.matmul(ps[:], wo[:, kk, :], attnT[:, kk, :],
                             start=(kk == 0), stop=(kk == 7))
        nc.vector.tensor_tensor(x2T[:, jt, :], ps[:], xTh[:, jt, 512:1024], OP.add)
    sO.close()

    if phases < 5:
        sM.close(); sA.close(); top.close(); return
    # ---------------- MLP ----------------
    # x2 natural layout via PE block transposes of x2T (exact up to fp32r)
    x2n = mpool.tile([128, 4, 1024], F32)
    ss2 = mpool.tile([128, 4], F32)
    scr2 = mpool.tile([128, 1024], F32)
    for qt in range(4):
        for jt in range(8):
            tp = psA.tile([128, W], F32R, tag="mm")
            nc.tensor.transpose(tp[:, 0:128], x2T[:, jt, 128 * qt:128 * (qt + 1)],
                                Id128r[:])
            nc.vector.tensor_copy(x2n[:, qt, 128 * jt:128 * (jt + 1)], tp[:, 0:128])
        nc.scalar.activation(scr2[:], x2n[:, qt, :], AF.Square,
                             accum_out=ss2[:, qt:qt + 1])
    nc.vector.tensor_scalar(rstd2sq[:], ss2[:], 1.0 / DIM, EPS, OP.mult, OP.add)
    nc.vector.reciprocal(rstd2sq[:], rstd2sq[:])

    outsb = mpool.tile([128, 4, 1024], F32)
    nc.vector.memset(outsb[:], 0.0)
    hp = sM.enter_context(tc.tile_pool(name="hp", bufs=10))
    w2p = sM.enter_context(tc.tile_pool(name="w2p", bufs=5))

    for g in range(4):
        hts = []
        for ml in range(8):
            mt = 8 * g + ml
            w1t = mw.tile([128, 8, 128], F32R, tag="w1t")
            nc.sync.dma_start(w1t[:], d["w1_tl"].ap()[mt])
            ps = psA.tile([128, W], F32, tag="mm")
            for kk in range(8):
                nc.tensor.matmul(ps[:], w1t[:, kk, :], x2T[:, kk, :],
                                 start=(kk == 0), stop=(kk == 7))
            ht = hp.tile([128, W], F32R, tag="ht", name=f"ht{g}_{ml}")
            nc.scalar.activation(ht[:], ps[:], AF.Relu)
            nc.vector.tensor_tensor(ht[:], ht[:], ht[:], OP.mult)
            hts.append(ht)
        w2ts = []
        for half in range(2):
            pss = [psD.tile([128, W], F32, tag="down", name=f"dn{g}_{half}_{i}")
                   for i in range(4)]
            for kl in range(8):
                if half == 0 and kl % 2 == 0:
                    w2t2 = w2p.tile([128, 2, 1024], F32R, tag="w2t", name=f"w2t{g}_{kl}")
                    nc.sync.dma_start(w2t2[:], d["w2_tl"].ap()[(8 * g + kl) // 2])
                    w2ts.append(w2t2)
                w2t = w2ts[kl // 2][:, kl % 2, :]
                for qt in range(4):
                    nc.tensor.matmul(pss[qt][:], hts[kl][:, 128 * qt:128 * (qt + 1)],
                                     w2t[:, 512 * half:512 * (half + 1)],
                                     start=(kl == 0), stop=(kl == 7))
            for qt in range(4):
                sl = slice(512 * half, 512 * (half + 1))
                if g < 3:
                    nc.vector.tensor_tensor(outsb[:, qt, sl], outsb[:, qt, sl],
                                            pss[qt][:], OP.add)
                else:
                    # last group: fold in the accumulated value, the mlp-norm
                    # scale and the residual, then ship this half out
                    nc.vector.tensor_tensor(outsb[:, qt, sl], outsb[:, qt, sl],
                                            pss[qt][:], OP.add)
                    nc.vector.scalar_tensor_tensor(
                        outsb[:, qt, sl], outsb[:, qt, sl], rstd2sq[:, qt:qt + 1],
                        x2n[:, qt, sl], OP.mult, OP.add)
                    nc.scalar.dma_start(
                        d["out"].ap()[128 * qt:128 * (qt + 1), sl], outsb[:, qt, sl])
    sM.close()
    sA.close()
    top.close()


def _build_program(phases=99, reps=1):
    import concourse.bacc as bacc
    import concourse.tile as tile
    import concourse.mybir as mybir

    F32 = mybir.dt.float32
    F32R = mybir.dt.float32r

    nc = bacc.Bacc("TRN2")
    d = {
        "xh": nc.declare_dram_parameter("xh", [1024, DIM], F32, isOutput=False),
        "xTh": nc.declare_dram_parameter("xTh", [128, 8, 1024], F32R, isOutput=False),
        "cosr": nc.declare_dram_parameter("cosr", [128, 1024], F32, isOutput=False),
        "sinr": nc.declare_dram_parameter("sinr", [128, 1024], F32, isOutput=False),
        "vpad": nc.declare_dram_parameter("vpad", [128, 8, 2], F32R, isOutput=False),
        "maskb": nc.declare_dram_parameter("maskb", [128, 8, W], mybir.dt.bfloat16, isOutput=False),
        "ones64": nc.declare_dram_parameter("ones64", [128, 64], F32R, isOutput=False),
        "Id128b": nc.declare_dram_parameter("Id128b", [128, 128], mybir.dt.bfloat16, isOutput=False),
        "E2": nc.declare_dram_parameter("E2", [128, 2], F32R, isOutput=False),
        "E2b": nc.declare_dram_parameter("E2b", [2, 128], F32R, isOutput=False),
        "Id128r": nc.declare_dram_parameter("Id128r", [128, 128], F32R, isOutput=False),
        "Pswap": nc.declare_dram_parameter("Pswap", [128, 128], F32R, isOutput=False),
        "qkv_wtl": nc.declare_dram_parameter("qkv_wtl", [16, 128, 8, 128], F32R, isOutput=False),
        "wv_tl": nc.declare_dram_parameter("wv_tl", [2, 128, 8, 512], F32R, isOutput=False),
        "wo_tl": nc.declare_dram_parameter("wo_tl", [8, 128, 8, 128], F32R, isOutput=False),
        "w1_tl": nc.declare_dram_parameter("w1_tl", [32, 128, 8, 128], F32R, isOutput=False),
        "w2_tl": nc.declare_dram_parameter("w2_tl", [16, 128, 2, 1024], F32R, isOutput=False),
        "out": nc.declare_dram_parameter("out", [W, DIM], F32, isOutput=True),
    }
    with tile.TileContext(nc) as tc:
        for _ in range(reps):
            _emit(nc, tc, d, mybir, phases=phases)
    nc.compile()
    return nc


def _host_prep(x, qkv_w, o_w, o_scale, w1, w2, w2_scale):
    x2d = np.ascontiguousarray(x.reshape(T, DIM).astype(np.float32))
    xT = np.ascontiguousarray(x2d.T)

    qkv_wT = qkv_w.astype(np.float32).T                                  # [1024, 3072]
    o_wsT = (o_w * o_scale[:, None]).astype(np.float32).T                # [1024, 1024]
    w1T = w1.astype(np.float32).T                                        # [1024, 4096]
    w2sT = (w2 * w2_scale[:, None]).astype(np.float32).T
    w2_tl = np.ascontiguousarray(
        w2sT.reshape(16, 2, 128, 1024).transpose(0, 2, 1, 3))

    def tile_cols(wT, ncols):
        # [1024, C] -> [C//ncols, 128, 8, ncols] with [ct, p, kk, m] layout
        kk8, p, ct = 8, 128, wT.shape[1] // ncols
        return np.ascontiguousarray(
            wT.reshape(kk8, p, ct, ncols).transpose(2, 1, 0, 3))

    qkv_wtl = tile_cols(qkv_wT[:, :2048], 128)        # [16, 128, 8, 128]
    wv_tl = tile_cols(qkv_wT[:, 2048:], 512)          # [2, 128, 8, 512]
    wo_tl = tile_cols(o_wsT, 128)                     # [8, 128, 8, 128]
    w1_tl = tile_cols(w1T, 128)                       # [32, 128, 8, 128]

    freqs = (1.0 / ROPE_BASE) ** np.linspace(0.0, 1.0, HD // 4, dtype=np.float32)
    freqs_d = np.concatenate([freqs, np.zeros(HD // 4, dtype=np.float32)])
    theta = np.arange(T, dtype=np.float32)[:, None] * freqs_d[None, :]   # [T, 32]
    cosT = np.cos(theta).T.astype(np.float32)                            # [32, T]
    sinT = np.sin(theta).T.astype(np.float32)
    cos_rep = np.concatenate([cosT, cosT, cosT, cosT], axis=0)           # [128, T]
    sin_sgn = np.concatenate([sinT, -sinT, sinT, -sinT], axis=0)

    in_maps = []
    for c in range(NCORES):
        r0 = c * W - W
        xh = np.zeros((1024, DIM), dtype=np.float32)
        xTh = np.zeros((DIM, 1024), dtype=np.float32)
        cosr = np.zeros((128, 1024), dtype=np.float32)
        sinr = np.zeros((128, 1024), dtype=np.float32)
        lo = max(r0, 0)
        off = lo - r0
        xh[off:1024] = x2d[lo:r0 + 1024]
        xTh[:, off:1024] = xT[:, lo:r0 + 1024]
        xTh_t = np.ascontiguousarray(xTh.reshape(8, 128, 1024).transpose(1, 0, 2))
        cosr[:, off:1024] = cos_rep[:, lo:r0 + 1024]
        sinr[:, off:1024] = sin_sgn[:, lo:r0 + 1024]
        import ml_dtypes
        vpadc = np.zeros((128, 8, 2), dtype=np.float32)
        vpadc[:, :, 0] = 1.0
        if c == 0:
            vpadc[:, 0:4, 0] = 0.0
        kvl = np.arange(1024)[:, None]
        ql = np.arange(W)[None, :]
        valid = (kvl > ql) & (kvl <= ql + W)
        mb = np.where(valid, 0.0, -240.0).astype(np.float32)
        maskbc = np.ascontiguousarray(
            mb.reshape(8, 128, W).transpose(1, 0, 2)).astype(ml_dtypes.bfloat16)
        E2c = (np.arange(128)[:, None] // 64 == np.arange(2)[None, :]).astype(np.float32)
        E2bc = np.ascontiguousarray(E2c.T)
        idc = np.eye(128, dtype=np.float32)
        psw = np.zeros((128, 128), dtype=np.float32)
        psw[np.arange(128), np.arange(128) ^ 32] = 1.0
        psw = np.ascontiguousarray(psw.T)  # lhsT layout: out[m] = in[m ^ 32]
        in_maps.append({
            "xh": xh,
            "xTh": xTh_t,
            "cosr": np.ascontiguousarray(cosr),
            "sinr": np.ascontiguousarray(sinr),
            "vpad": vpadc,
            "maskb": maskbc,
            "ones64": np.ones((128, 64), dtype=np.float32),
            "Id128b": np.eye(128, dtype=np.float32).astype(ml_dtypes.bfloat16),
            "E2": E2c,
            "E2b": E2bc,
            "Id128r": idc,
            "Pswap": psw,
            "qkv_wtl": qkv_wtl,
            "wv_tl": wv_tl,
            "wo_tl": wo_tl,
            "w1_tl": w1_tl,
            "w2_tl": w2_tl,
        })
    return in_maps


def kernel(x, qkv_w, o_w, o_scale, w1, w2, w2_scale):
    from concourse.bass_utils import run_bass_kernel_spmd

    global _program
    with _lock:
        if _program is None:
            _program = _build_program()
    in_maps = _host_prep(x, qkv_w, o_w, o_scale, w1, w2, w2_scale)
    res = run_bass_kernel_spmd(_program, in_maps, core_ids=list(range(NCORES)))
    out = np.concatenate([res.results[c]["out"] for c in range(NCORES)], axis=0)
    return out.reshape(1, T, DIM).astype(np.float32)
